# revision 7
# baseline (speedup 1.0000x reference)
"""Trainium2 Bass kernel for nn_AgesRRN (batched 8-node GNN message passing).

Strategy (pure data parallel over 8 cores, 256 graphs each):
  - Activations live transposed in SBUF: [128 feature partitions x 2048 node cols],
    stored fp16 (values are O(1e-2); fp16 keeps ~5e-4 relative accuracy end to
    end while doubling DVE throughput and running the PE at 1 cycle/row).
  - Edges are ALL ordered pairs (i,j), i!=j inside each 8-node graph, so the
    edge-MLP layer 1 splits as u_i + v'_j with u = A^T h, v' = B^T h + b1
    (A/B = msg_w1 row blocks; the edge-feature rows multiply zeros).
  - relu(u+v') == max(u, -v') - (-v'), so with W := -v' the aggregated message
    S_i = sum_{j!=i} relu(u_i + v'_j)
        = sum_j max(u_i, W_j) - max(u_i, W_i) + W_i - sum_j W_j
    The max table is one DVE op per chunk (GPSIMD pre-expands u along j so both
    operands stream at the DVE 2x fp16 rate), reduced over j by a 3-level
    tensor_tensor adder tree; every linear term is folded into the
    PSUM-accumulated m_agg matmuls with +/-msg_w2.
  - sum_j W_j comes from the per-graph pooled h (already needed for the logits
    head): sum_j v'_j = B^T pooled_h + 8 b1.
  - Node MLP: psum = C2^T h + C3^T m_agg + I @ xc (xc = C1^T x + node_b1 is
    step-invariant), relu on ACT, then layer 2. Logits head on pooled h_new.
"""

import sys

if "/opt/trn_rl_repo" not in sys.path:
    sys.path.insert(0, "/opt/trn_rl_repo")

from contextlib import ExitStack

import numpy as np

import concourse.bass as bass  # noqa: F401
import concourse.mybir as mybir
import concourse.tile as tile
from concourse import bacc
from concourse.bass_utils import run_bass_kernel_spmd
from concourse.masks import make_identity

N_CORES = 8
BS = 2048
N_NODES = 8
N_STEPS = 8
G = BS // N_CORES          # graphs per core
NCOL = G * N_NODES         # node columns per core
CH = 4                     # chunks per step
CW = NCOL // CH            # node cols per chunk (512)
GW = G // CH               # graphs per chunk (64)
PW = GW * 64               # pair cols per chunk (4096)
F32 = mybir.dt.float32
F16 = mybir.dt.float16
AF = mybir.ActivationFunctionType
ALU = mybir.AluOpType

_W_NAMES = [
    ("pre_w1", [128, 128]),
    ("pre_w2", [128, 128]),
    ("msg_a", [128, 128]),
    ("msg_b", [128, 128]),
    ("msg_w2", [128, 128]),
    ("c1", [128, 128]),
    ("c2", [128, 128]),
    ("c3", [128, 128]),
    ("node_w2", [128, 128]),
    ("out_w1", [128, 128]),
    ("out_w2", [128, 100]),
]
_B_NAMES = [
    ("pre_b1", 128),
    ("pre_b2", 128),
    ("msg_b1", 128),
    ("msg_b2", 128),
    ("node_b1", 128),
    ("node_b2", 128),
    ("out_b1", 128),
    ("out_b2", 100),
]


def _kernel_body(tc, d):
    nc = tc.nc
    mm = nc.tensor.matmul
    with ExitStack() as ctx:
        wp = ctx.enter_context(tc.tile_pool(name="wp", bufs=1))
        sp = ctx.enter_context(tc.tile_pool(name="sp", bufs=1))
        rot = ctx.enter_context(tc.tile_pool(name="rot", bufs=2))
        mxp = ctx.enter_context(tc.tile_pool(name="mxp", bufs=2))
        ps = ctx.enter_context(tc.tile_pool(name="ps", bufs=1, space="PSUM"))

        # ---- resident weights (fp16) / biases (fp32) -----------------------
        w = {}
        for name, shape in _W_NAMES:
            w[name] = wp.tile(shape, F16, name=f"w_{name}", tag=f"w_{name}")
            nc.sync.dma_start(w[name][:], d[name].ap())
        b = {}
        for name, n in _B_NAMES:
            b[name] = wp.tile([128, 1], F32, name=f"b_{name}", tag=f"b_{name}")
            nc.sync.dma_start(b[name][:n, :], d[name].ap()[:, None])

        ident = wp.tile([128, 128], F16, name="ident", tag="ident")
        make_identity(nc, ident[:])

        # derived small constants
        w2n = wp.tile([128, 128], F16, name="w2n", tag="w2n")
        nc.scalar.mul(w2n[:], w["msg_w2"][:], -1.0)
        negb1 = wp.tile([128, 1], F32, name="negb1", tag="negb1")
        nc.scalar.mul(negb1[:], b["msg_b1"][:], -1.0)
        b1x8 = wp.tile([128, 1], F32, name="b1x8", tag="b1x8")
        nc.scalar.mul(b1x8[:], b["msg_b1"][:], 8.0)
        b2x7 = wp.tile([128, 1], F32, name="b2x7", tag="b2x7")
        nc.scalar.mul(b2x7[:], b["msg_b2"][:], 7.0)

        iota_i = wp.tile([128, 1], mybir.dt.int32, name="iota_i", tag="iota_i")
        nc.gpsimd.iota(iota_i[:], pattern=[[0, 1]], base=0, channel_multiplier=1)
        iota_f = wp.tile([128, 1], F32, name="iota_f", tag="iota_f")
        nc.vector.tensor_copy(iota_f[:], iota_i[:])

        # ---- pre phase: one-hot features + pre-MLP -------------------------
        idxb = sp.tile([128, NCOL], F32, name="idxb", tag="idxb")
        nc.sync.dma_start(idxb[:], d["idxb"].ap())
        moh = sp.tile([128, NCOL], F16, name="moh", tag="moh")
        l1 = sp.tile([128, NCOL], F16, name="l1", tag="l1")
        x = sp.tile([128, NCOL], F16, name="x", tag="x")
        xc = sp.tile([128, NCOL], F16, name="xc", tag="xc")
        for c in range(CH):
            cs = slice(c * CW, (c + 1) * CW)
            nc.vector.tensor_scalar(
                moh[:, cs], idxb[:, cs], iota_f[:], None, op0=ALU.is_equal
            )
            pp = ps.tile([128, CW], F32, name="ps", tag="ps", bufs=6)
            mm(out=pp[:], lhsT=w["pre_w1"][:], rhs=moh[:, cs])
            nc.scalar.activation(l1[:, cs], pp[:], AF.Relu, bias=b["pre_b1"][:])
            pp2 = ps.tile([128, CW], F32, name="ps", tag="ps", bufs=6)
            mm(out=pp2[:], lhsT=w["pre_w2"][:], rhs=l1[:, cs])
            nc.scalar.activation(x[:, cs], pp2[:], AF.Identity, bias=b["pre_b2"][:])
            pp3 = ps.tile([128, CW], F32, name="ps", tag="ps", bufs=6)
            mm(out=pp3[:], lhsT=w["c1"][:], rhs=x[:, cs])
            nc.scalar.activation(xc[:, cs], pp3[:], AF.Identity, bias=b["node_b1"][:])

        pooled = rot.tile([128, G], F16, name="pooled", tag="pooled")
        with nc.allow_low_precision("fp16 pooled sum of 8"):
            nc.vector.reduce_sum(
                pooled[:], x[:].rearrange("p (g i) -> p g i", i=N_NODES),
                axis=mybir.AxisListType.X,
            )

        hbufs = [sp.tile([128, NCOL], F16, name=f"h{k}", tag=f"h{k}") for k in range(2)]

        # ---- recurrent steps ----------------------------------------------
        for s in range(N_STEPS):
            h = x if s == 0 else hbufs[(s + 1) % 2]
            hn = hbufs[s % 2]
            u = sp.tile([128, NCOL], F16, name="u", tag="u")
            wt = sp.tile([128, NCOL], F16, name="wt", tag="wt")
            r = sp.tile([128, NCOL], F16, name="r", tag="r")
            md = sp.tile([128, NCOL], F16, name="md", tag="md")
            magg = sp.tile([128, NCOL], F16, name="magg", tag="magg")
            nl1 = sp.tile([128, NCOL], F16, name="nl1", tag="nl1")

            # u = A^T h ; W = -(B^T h + b1)
            for c in range(CH):
                cs = slice(c * CW, (c + 1) * CW)
                pu = ps.tile([128, CW], F32, name="ps", tag="ps", bufs=6)
                mm(out=pu[:], lhsT=w["msg_a"][:], rhs=h[:, cs])
                nc.scalar.copy(u[:, cs], pu[:])
                pw = ps.tile([128, CW], F32, name="ps", tag="ps", bufs=6)
                mm(out=pw[:], lhsT=w["msg_b"][:], rhs=h[:, cs])
                nc.scalar.activation(
                    wt[:, cs], pw[:], AF.Identity, scale=-1.0, bias=negb1[:]
                )

            # Tpos = B^T pooled_h + 8 b1  (= sum_j v'_j per graph)
            pt = ps.tile([128, G], F32, name="pst", tag="pst", bufs=2)
            mm(out=pt[:], lhsT=w["msg_b"][:], rhs=pooled[:])
            tpos = rot.tile([128, G], F16, name="tpos", tag="tpos")
            nc.scalar.activation(tpos[:], pt[:], AF.Identity, bias=b1x8[:])

            # pairwise max + adder-tree reduce over j ; diagonal max
            u3 = u[:].rearrange("p (g i) -> p g i", i=N_NODES)
            w3 = wt[:].rearrange("p (g j) -> p g j", j=N_NODES)
            for c in range(CH):
                gs = slice(c * GW, (c + 1) * GW)
                cs = slice(c * CW, (c + 1) * CW)
                # GPSIMD expands u_i across j so the DVE max runs at 2x
                urep = mxp.tile([128, GW, N_NODES, N_NODES], F16,
                                name="urep", tag="urep")
                ub = u3[:, gs, :, None].to_broadcast([128, GW, N_NODES, N_NODES])
                nc.gpsimd.tensor_copy(urep[:], ub)
                wb = w3[:, gs, None, :].to_broadcast([128, GW, N_NODES, N_NODES])
                mx = mxp.tile([128, GW, N_NODES, N_NODES], F16, name="mx", tag="mx")
                nc.vector.tensor_tensor(mx[:], urep[:], wb, op=ALU.max)
                t1 = mxp.tile([128, GW, N_NODES, 4], F16, name="t1", tag="t1")
                nc.vector.tensor_add(t1[:], mx[:, :, :, 0:4], mx[:, :, :, 4:8])
                t2 = mxp.tile([128, GW, N_NODES, 2], F16, name="t2", tag="t2")
                nc.vector.tensor_add(t2[:], t1[:, :, :, 0:2], t1[:, :, :, 2:4])
                nc.vector.tensor_add(
                    r[:, cs].rearrange("p (g i) -> p g i", i=N_NODES),
                    t2[:, :, :, 0], t2[:, :, :, 1],
                )
                nc.vector.tensor_max(md[:, cs], u[:, cs], wt[:, cs])

            # m_agg = W2^T (R - MD + W + Tpos_bcast) + 7 b2
            for c in range(CH):
                gs = slice(c * GW, (c + 1) * GW)
                cs = slice(c * CW, (c + 1) * CW)
                pm = ps.tile([128, CW], F32, name="ps", tag="ps", bufs=6)
                mm(out=pm[:], lhsT=w["msg_w2"][:], rhs=r[:, cs],
                   start=True, stop=False)
                mm(out=pm[:], lhsT=w2n[:], rhs=md[:, cs], start=False, stop=False)
                mm(out=pm[:], lhsT=w["msg_w2"][:], rhs=wt[:, cs],
                   start=False, stop=False)
                tb = tpos[:][:, gs][:, :, None].to_broadcast([128, GW, N_NODES])
                mm(out=pm[:], lhsT=w["msg_w2"][:], rhs=tb, start=False, stop=True)
                nc.scalar.activation(magg[:, cs], pm[:], AF.Identity, bias=b2x7[:])

                # node MLP layer 1: C2^T h + C3^T m_agg + xc, relu
                pn = ps.tile([128, CW], F32, name="ps", tag="ps", bufs=6)
                mm(out=pn[:], lhsT=w["c2"][:], rhs=h[:, cs], start=True, stop=False)
                mm(out=pn[:], lhsT=w["c3"][:], rhs=magg[:, cs],
                   start=False, stop=False)
                mm(out=pn[:], lhsT=ident[:], rhs=xc[:, cs], start=False, stop=True)
                nc.scalar.activation(nl1[:, cs], pn[:], AF.Relu)

                # node MLP layer 2
                ph = ps.tile([128, CW], F32, name="ps", tag="ps", bufs=6)
                mm(out=ph[:], lhsT=w["node_w2"][:], rhs=nl1[:, cs])
                nc.scalar.activation(
                    hn[:, cs], ph[:], AF.Identity, bias=b["node_b2"][:]
                )

            pooled = rot.tile([128, G], F16, name="pooled", tag="pooled")
            with nc.allow_low_precision("fp16 pooled sum of 8"):
                nc.vector.reduce_sum(
                    pooled[:], hn[:].rearrange("p (g i) -> p g i", i=N_NODES),
                    axis=mybir.AxisListType.X,
                )

            # logits head on pooled h_new
            po1 = ps.tile([128, G], F32, name="pst", tag="pst", bufs=2)
            mm(out=po1[:], lhsT=w["out_w1"][:], rhs=pooled[:])
            o1 = rot.tile([128, G], F16, name="o1", tag="o1")
            nc.scalar.activation(o1[:], po1[:], AF.Relu, bias=b["out_b1"][:])
            po2 = ps.tile([128, G], F32, name="pst", tag="pst", bufs=2)
            mm(out=po2[:100], lhsT=w["out_w2"][:], rhs=o1[:])
            olog = rot.tile([128, G], F32, name="olog", tag="olog")
            nc.scalar.activation(
                olog[:100, :], po2[:100], AF.Identity, bias=b["out_b2"][:100]
            )
            nc.sync.dma_start(d["out"].ap()[s], olog[:100, :])


def build_nc():
    nc = bacc.Bacc(
        "TRN2",
        target_bir_lowering=False,
        debug=False,
        enable_asserts=True,
        num_devices=N_CORES,
    )
    d = {}
    d["idxb"] = nc.dram_tensor("idxb", [128, NCOL], F32, kind="ExternalInput")
    for name, shape in _W_NAMES:
        d[name] = nc.dram_tensor(name, shape, F16, kind="ExternalInput")
    for name, n in _B_NAMES:
        d[name] = nc.dram_tensor(name, [n], F32, kind="ExternalInput")
    d["out"] = nc.dram_tensor(
        "out", [N_STEPS, 100, G], F32, kind="ExternalOutput"
    )
    with tile.TileContext(nc) as tc:
        _kernel_body(tc, d)
    nc.compile()
    return nc


def prep_inputs(**inputs):
    """Host-side layout prep: shard + transpose/slice/cast. Returns in_maps."""
    f32 = np.float32
    f16 = np.float16
    src = np.asarray(inputs["sources"]).astype(f32)        # (BS, 8)
    tgt = np.asarray(inputs["targets"]).astype(f32)
    typ = np.asarray(inputs["types"]).astype(f32)
    dif = np.asarray(inputs["diffs"]).astype(f32)
    q = np.asarray(inputs["question"]).astype(f32)         # (BS,)
    qrep = np.repeat(q[:, None], N_NODES, axis=1)

    shared = {}
    pre_w1 = np.asarray(inputs["pre_w1"], dtype=f32)
    pre_w1p = np.zeros((128, 128), f32)
    pre_w1p[:127] = pre_w1
    shared["pre_w1"] = pre_w1p
    shared["pre_w2"] = np.asarray(inputs["pre_w2"], dtype=f32)
    msg_w1 = np.asarray(inputs["msg_w1"], dtype=f32)
    shared["msg_a"] = msg_w1[0:128]
    shared["msg_b"] = msg_w1[128:256]
    shared["msg_w2"] = np.asarray(inputs["msg_w2"], dtype=f32)
    node_w1 = np.asarray(inputs["node_w1"], dtype=f32)
    shared["c1"] = node_w1[0:128]
    shared["c2"] = node_w1[128:256]
    shared["c3"] = node_w1[256:384]
    shared["node_w2"] = np.asarray(inputs["node_w2"], dtype=f32)
    shared["out_w1"] = np.asarray(inputs["out_w1"], dtype=f32)
    shared["out_w2"] = np.asarray(inputs["out_w2"], dtype=f32)
    shared = {k: np.ascontiguousarray(v, dtype=f16) for k, v in shared.items()}
    for name, _n in _B_NAMES:
        shared[name] = np.ascontiguousarray(inputs[name], dtype=f32)

    # per-partition-group index rows (offsets into the 127 one-hot rows)
    rows = [src, 8.0 + tgt, 16.0 + typ, 19.0 + dif, 119.0 + qrep]
    bounds = [(0, 8), (8, 16), (16, 19), (19, 119), (119, 127)]
    in_maps = []
    for c in range(N_CORES):
        gsl = slice(c * G, (c + 1) * G)
        idxb = np.full((128, NCOL), -1.0, f32)
        for (lo, hi), row in zip(bounds, rows):
            idxb[lo:hi, :] = row[gsl].reshape(-1)[None, :]
        m = dict(shared)
        m["idxb"] = idxb
        in_maps.append(m)
    return in_maps


_NC_CACHE = {}


def kernel(**inputs) -> np.ndarray:
    if "nc" not in _NC_CACHE:
        _NC_CACHE["nc"] = build_nc()
    nc = _NC_CACHE["nc"]
    in_maps = prep_inputs(**inputs)
    res = run_bass_kernel_spmd(nc, in_maps, list(range(N_CORES)))
    parts = [res.results[c]["out"].transpose(0, 2, 1) for c in range(N_CORES)]
    return np.ascontiguousarray(
        np.concatenate(parts, axis=1), dtype=np.float32
    )


# revision 9
# speedup vs baseline: 1.9493x; 1.9493x over previous
"""Trainium2 Bass kernel for nn_AgesRRN (batched 8-node GNN message passing).

Strategy (pure data parallel over 8 cores, 256 graphs each):
  - Activations live transposed in SBUF: [128 feature partitions x 2048 node cols],
    stored fp16 (values are O(1e-2); fp16 keeps ~5e-4 relative accuracy end to
    end while doubling DVE throughput and running the PE at 1 cycle/row).
  - Edges are ALL ordered pairs (i,j), i!=j inside each 8-node graph, so the
    edge-MLP layer 1 splits as u_i + v'_j with u = A^T h, v' = B^T h + b1
    (A/B = msg_w1 row blocks; the edge-feature rows multiply zeros).
  - relu(u+v') == max(u, -v') - (-v'), so with W := -v' the aggregated message
    S_i = sum_{j!=i} relu(u_i + v'_j)
        = sum_j max(u_i, W_j) - max(u_i, W_i) + W_i - sum_j W_j
    The max table is one DVE op per chunk (GPSIMD pre-expands u along j so both
    operands stream at the DVE 2x fp16 rate), reduced over j by a 3-level
    tensor_tensor adder tree; every linear term is folded into the
    PSUM-accumulated m_agg matmuls with +/-msg_w2.
  - sum_j W_j comes from the per-graph pooled h (already needed for the logits
    head): sum_j v'_j = B^T pooled_h + 8 b1.
  - Node MLP: psum = C2^T h + C3^T m_agg + I @ xc (xc = C1^T x + node_b1 is
    step-invariant), relu on ACT, then layer 2. Logits head on pooled h_new.
"""

import sys

if "/opt/trn_rl_repo" not in sys.path:
    sys.path.insert(0, "/opt/trn_rl_repo")

from contextlib import ExitStack

import numpy as np

import concourse.bass as bass  # noqa: F401
import concourse.mybir as mybir
import concourse.tile as tile
from concourse import bacc
from concourse.bass_utils import run_bass_kernel_spmd
from concourse.masks import make_identity

N_CORES = 8
BS = 2048
N_NODES = 8
N_STEPS = 8
G = BS // N_CORES          # graphs per core
NCOL = G * N_NODES         # node columns per core
CH = 4                     # chunks per step
CW = NCOL // CH            # node cols per chunk (512)
GW = G // CH               # graphs per chunk (64)
PW = GW * 64               # pair cols per chunk (4096)
F32 = mybir.dt.float32
F16 = mybir.dt.float16
AF = mybir.ActivationFunctionType
ALU = mybir.AluOpType

_W_NAMES = [
    ("pre_w1", [128, 128]),
    ("pre_w2", [128, 128]),
    ("msg_a", [128, 128]),
    ("msg_b", [128, 128]),
    ("msg_w2", [128, 128]),
    ("c1", [128, 128]),
    ("c2", [128, 128]),
    ("c3", [128, 128]),
    ("node_w2", [128, 128]),
    ("out_w1", [128, 128]),
    ("out_w2", [128, 100]),
]
_B_NAMES = [
    ("pre_b1", 128),
    ("pre_b2", 128),
    ("msg_b1", 128),
    ("msg_b2", 128),
    ("node_b1", 128),
    ("node_b2", 128),
    ("out_b1", 128),
    ("out_b2", 100),
]


def _kernel_body(tc, d):
    nc = tc.nc
    mm = nc.tensor.matmul
    with ExitStack() as ctx:
        wp = ctx.enter_context(tc.tile_pool(name="wp", bufs=1))
        sp = ctx.enter_context(tc.tile_pool(name="sp", bufs=1))
        rot = ctx.enter_context(tc.tile_pool(name="rot", bufs=2))
        mxp = ctx.enter_context(tc.tile_pool(name="mxp", bufs=2))
        ps = ctx.enter_context(tc.tile_pool(name="ps", bufs=1, space="PSUM"))

        # ---- resident weights (fp16) / biases (fp32) -----------------------
        w = {}
        for name, shape in _W_NAMES:
            w[name] = wp.tile(shape, F16, name=f"w_{name}", tag=f"w_{name}")
            nc.sync.dma_start(w[name][:], d[name].ap())
        b = {}
        for name, n in _B_NAMES:
            b[name] = wp.tile([128, 1], F32, name=f"b_{name}", tag=f"b_{name}")
            nc.sync.dma_start(b[name][:n, :], d[name].ap()[:, None])

        ident = wp.tile([128, 128], F16, name="ident", tag="ident")
        make_identity(nc, ident[:])

        # derived small constants
        w2n = wp.tile([128, 128], F16, name="w2n", tag="w2n")
        nc.scalar.mul(w2n[:], w["msg_w2"][:], -1.0)
        b2x7 = wp.tile([128, 1], F32, name="b2x7", tag="b2x7")
        nc.scalar.mul(b2x7[:], b["msg_b2"][:], 7.0)

        iota_i = wp.tile([128, 1], mybir.dt.int32, name="iota_i", tag="iota_i")
        nc.gpsimd.iota(iota_i[:], pattern=[[0, 1]], base=0, channel_multiplier=1)
        iota_f = wp.tile([128, 1], F32, name="iota_f", tag="iota_f")
        nc.vector.tensor_copy(iota_f[:], iota_i[:])

        # ---- pre phase: one-hot features + pre-MLP -------------------------
        idxb = sp.tile([128, NCOL], F32, name="idxb", tag="idxb")
        nc.sync.dma_start(idxb[:], d["idxb"].ap())
        moh = sp.tile([128, NCOL], F16, name="moh", tag="moh")
        l1 = sp.tile([128, NCOL], F16, name="l1", tag="l1")
        x = sp.tile([128, NCOL], F16, name="x", tag="x")
        xc = sp.tile([128, NCOL], F16, name="xc", tag="xc")
        for c in range(CH):
            cs = slice(c * CW, (c + 1) * CW)
            nc.vector.tensor_scalar(
                moh[:, cs], idxb[:, cs], iota_f[:], None, op0=ALU.is_equal
            )
            pp = ps.tile([128, CW], F32, name="ps", tag="ps", bufs=4)
            mm(out=pp[:], lhsT=w["pre_w1"][:], rhs=moh[:, cs])
            nc.scalar.activation(l1[:, cs], pp[:], AF.Relu, bias=b["pre_b1"][:])
            pp2 = ps.tile([128, CW], F32, name="ps", tag="ps", bufs=4)
            mm(out=pp2[:], lhsT=w["pre_w2"][:], rhs=l1[:, cs])
            nc.scalar.activation(x[:, cs], pp2[:], AF.Identity, bias=b["pre_b2"][:])
            pp3 = ps.tile([128, CW], F32, name="ps", tag="ps", bufs=4)
            mm(out=pp3[:], lhsT=w["c1"][:], rhs=x[:, cs])
            nc.scalar.activation(xc[:, cs], pp3[:], AF.Identity, bias=b["node_b1"][:])

        pooled = rot.tile([128, G], F16, name="pooled", tag="pooled")
        with nc.allow_low_precision("fp16 pooled sum of 8"):
            nc.vector.reduce_sum(
                pooled[:], x[:].rearrange("p (g i) -> p g i", i=N_NODES),
                axis=mybir.AxisListType.X,
            )

        hbufs = [sp.tile([128, NCOL], F16, name=f"h{k}", tag=f"h{k}") for k in range(2)]

        # ---- recurrent steps ----------------------------------------------
        # Pairwise pre-activations z_ij = u_i + v'_j are built by the PE
        # itself: two PSUM-accumulated matmuls whose rhs APs broadcast the
        # node columns across j (A side) / across i (B side). relu(+b1) then
        # lands the pair table in SBUF fp16 (split across ACT and DVE), and a
        # 3-op adder tree reduces over j.
        GZ = 16                      # graphs per z-psum tile (1024 pairs)
        NZT = G // GZ                # z tiles per step (16)
        for s in range(N_STEPS):
            h = x if s == 0 else hbufs[(s + 1) % 2]
            hn = hbufs[s % 2]
            r = sp.tile([128, NCOL], F16, name="r", tag="r")
            mdr = sp.tile([128, NCOL], F16, name="mdr", tag="mdr")
            magg = sp.tile([128, NCOL], F16, name="magg", tag="magg")
            nl1 = sp.tile([128, NCOL], F16, name="nl1", tag="nl1")
            mx = mxp.tile([128, NCOL * N_NODES], F16, name="mx", tag="mx")
            h3 = h[:].rearrange("p (g i) -> p g i", i=N_NODES)

            for t in range(NZT):
                zp = ps.tile([128, 2 * CW], F32, name="zps", tag="zps", bufs=2)
                for q in range(2):
                    g0 = t * GZ + q * 8
                    rhs_a = h3[:, g0:g0 + 8, :, None].to_broadcast(
                        [128, 8, N_NODES, N_NODES])
                    rhs_b = h3[:, g0:g0 + 8, None, :].to_broadcast(
                        [128, 8, N_NODES, N_NODES])
                    zsl = zp[:, q * CW:(q + 1) * CW]
                    mm(out=zsl, lhsT=w["msg_a"][:], rhs=rhs_a,
                       start=True, stop=False)
                    mm(out=zsl, lhsT=w["msg_b"][:], rhs=rhs_b,
                       start=False, stop=True)
                msl = mx[:, t * 2 * CW:(t + 1) * 2 * CW]
                if t % 2 == 0:
                    nc.scalar.activation(msl, zp[:], AF.Relu, bias=b["msg_b1"][:])
                else:
                    nc.vector.tensor_scalar(
                        msl, zp[:], b["msg_b1"][:], 0.0, op0=ALU.add, op1=ALU.max
                    )

            # diagonal term relu(u_i + v'_i) and adder tree over j
            for c in range(CH):
                cs = slice(c * CW, (c + 1) * CW)
                zd = ps.tile([128, CW], F32, name="ps", tag="ps", bufs=4)
                mm(out=zd[:], lhsT=w["msg_a"][:], rhs=h[:, cs],
                   start=True, stop=False)
                mm(out=zd[:], lhsT=w["msg_b"][:], rhs=h[:, cs],
                   start=False, stop=True)
                nc.scalar.activation(mdr[:, cs], zd[:], AF.Relu, bias=b["msg_b1"][:])

            mx3 = mx[:].rearrange("p (n j) -> p n j", j=N_NODES)
            t1 = mxp.tile([128, NCOL, 4], F16, name="t1", tag="t1")
            nc.vector.tensor_add(t1[:], mx3[:, :, 0:4], mx3[:, :, 4:8])
            t2 = mxp.tile([128, NCOL, 2], F16, name="t2", tag="t2")
            nc.vector.tensor_add(t2[:], t1[:, :, 0:2], t1[:, :, 2:4])
            nc.vector.tensor_add(
                r[:].rearrange("p (n one) -> p n one", one=1),
                t2[:, :, 0:1], t2[:, :, 1:2],
            )

            # m_agg = W2^T (R - diag) + 7 b2
            for c in range(CH):
                cs = slice(c * CW, (c + 1) * CW)
                pm = ps.tile([128, CW], F32, name="ps", tag="ps", bufs=4)
                mm(out=pm[:], lhsT=w["msg_w2"][:], rhs=r[:, cs],
                   start=True, stop=False)
                mm(out=pm[:], lhsT=w2n[:], rhs=mdr[:, cs], start=False, stop=True)
                nc.scalar.activation(magg[:, cs], pm[:], AF.Identity, bias=b2x7[:])

                # node MLP layer 1: C2^T h + C3^T m_agg + xc, relu
                pn = ps.tile([128, CW], F32, name="ps", tag="ps", bufs=4)
                mm(out=pn[:], lhsT=w["c2"][:], rhs=h[:, cs], start=True, stop=False)
                mm(out=pn[:], lhsT=w["c3"][:], rhs=magg[:, cs],
                   start=False, stop=False)
                mm(out=pn[:], lhsT=ident[:], rhs=xc[:, cs], start=False, stop=True)
                nc.scalar.activation(nl1[:, cs], pn[:], AF.Relu)

                # node MLP layer 2
                ph = ps.tile([128, CW], F32, name="ps", tag="ps", bufs=4)
                mm(out=ph[:], lhsT=w["node_w2"][:], rhs=nl1[:, cs])
                nc.scalar.activation(
                    hn[:, cs], ph[:], AF.Identity, bias=b["node_b2"][:]
                )

            pooled = rot.tile([128, G], F16, name="pooled", tag="pooled")
            with nc.allow_low_precision("fp16 pooled sum of 8"):
                nc.vector.reduce_sum(
                    pooled[:], hn[:].rearrange("p (g i) -> p g i", i=N_NODES),
                    axis=mybir.AxisListType.X,
                )

            # logits head on pooled h_new
            po1 = ps.tile([128, G], F32, name="pst", tag="ps", bufs=4)
            mm(out=po1[:], lhsT=w["out_w1"][:], rhs=pooled[:])
            o1 = rot.tile([128, G], F16, name="o1", tag="o1")
            nc.scalar.activation(o1[:], po1[:], AF.Relu, bias=b["out_b1"][:])
            po2 = ps.tile([128, G], F32, name="pst", tag="ps", bufs=4)
            mm(out=po2[:100], lhsT=w["out_w2"][:], rhs=o1[:])
            olog = rot.tile([128, G], F32, name="olog", tag="olog")
            nc.scalar.activation(
                olog[:100, :], po2[:100], AF.Identity, bias=b["out_b2"][:100]
            )
            nc.sync.dma_start(d["out"].ap()[s], olog[:100, :])


def build_nc():
    nc = bacc.Bacc(
        "TRN2",
        target_bir_lowering=False,
        debug=False,
        enable_asserts=True,
        num_devices=N_CORES,
    )
    d = {}
    d["idxb"] = nc.dram_tensor("idxb", [128, NCOL], F32, kind="ExternalInput")
    for name, shape in _W_NAMES:
        d[name] = nc.dram_tensor(name, shape, F16, kind="ExternalInput")
    for name, n in _B_NAMES:
        d[name] = nc.dram_tensor(name, [n], F32, kind="ExternalInput")
    d["out"] = nc.dram_tensor(
        "out", [N_STEPS, 100, G], F32, kind="ExternalOutput"
    )
    with tile.TileContext(nc) as tc:
        _kernel_body(tc, d)
    nc.compile()
    return nc


def prep_inputs(**inputs):
    """Host-side layout prep: shard + transpose/slice/cast. Returns in_maps."""
    f32 = np.float32
    f16 = np.float16
    src = np.asarray(inputs["sources"]).astype(f32)        # (BS, 8)
    tgt = np.asarray(inputs["targets"]).astype(f32)
    typ = np.asarray(inputs["types"]).astype(f32)
    dif = np.asarray(inputs["diffs"]).astype(f32)
    q = np.asarray(inputs["question"]).astype(f32)         # (BS,)
    qrep = np.repeat(q[:, None], N_NODES, axis=1)

    shared = {}
    pre_w1 = np.asarray(inputs["pre_w1"], dtype=f32)
    pre_w1p = np.zeros((128, 128), f32)
    pre_w1p[:127] = pre_w1
    shared["pre_w1"] = pre_w1p
    shared["pre_w2"] = np.asarray(inputs["pre_w2"], dtype=f32)
    msg_w1 = np.asarray(inputs["msg_w1"], dtype=f32)
    shared["msg_a"] = msg_w1[0:128]
    shared["msg_b"] = msg_w1[128:256]
    shared["msg_w2"] = np.asarray(inputs["msg_w2"], dtype=f32)
    node_w1 = np.asarray(inputs["node_w1"], dtype=f32)
    shared["c1"] = node_w1[0:128]
    shared["c2"] = node_w1[128:256]
    shared["c3"] = node_w1[256:384]
    shared["node_w2"] = np.asarray(inputs["node_w2"], dtype=f32)
    shared["out_w1"] = np.asarray(inputs["out_w1"], dtype=f32)
    shared["out_w2"] = np.asarray(inputs["out_w2"], dtype=f32)
    shared = {k: np.ascontiguousarray(v, dtype=f16) for k, v in shared.items()}
    for name, _n in _B_NAMES:
        shared[name] = np.ascontiguousarray(inputs[name], dtype=f32)

    # per-partition-group index rows (offsets into the 127 one-hot rows)
    rows = [src, 8.0 + tgt, 16.0 + typ, 19.0 + dif, 119.0 + qrep]
    bounds = [(0, 8), (8, 16), (16, 19), (19, 119), (119, 127)]
    in_maps = []
    for c in range(N_CORES):
        gsl = slice(c * G, (c + 1) * G)
        idxb = np.full((128, NCOL), -1.0, f32)
        for (lo, hi), row in zip(bounds, rows):
            idxb[lo:hi, :] = row[gsl].reshape(-1)[None, :]
        m = dict(shared)
        m["idxb"] = idxb
        in_maps.append(m)
    return in_maps


_NC_CACHE = {}


def kernel(**inputs) -> np.ndarray:
    if "nc" not in _NC_CACHE:
        _NC_CACHE["nc"] = build_nc()
    nc = _NC_CACHE["nc"]
    in_maps = prep_inputs(**inputs)
    res = run_bass_kernel_spmd(nc, in_maps, list(range(N_CORES)))
    parts = [res.results[c]["out"].transpose(0, 2, 1) for c in range(N_CORES)]
    return np.ascontiguousarray(
        np.concatenate(parts, axis=1), dtype=np.float32
    )


# revision 11
# speedup vs baseline: 2.8616x; 1.4680x over previous
"""Trainium2 Bass kernel for nn_AgesRRN (batched 8-node GNN message passing).

Strategy (pure data parallel over 8 cores, 256 graphs each):
  - Activations live transposed in SBUF: [128 feature partitions x 2048 node cols],
    stored fp16 (values are O(1e-2); fp16 keeps ~5e-4 relative accuracy end to
    end while doubling DVE throughput and running the PE at 1 cycle/row).
  - Edges are ALL ordered pairs (i,j), i!=j inside each 8-node graph, so the
    edge-MLP layer 1 splits as u_i + v'_j with u = A^T h, v' = B^T h + b1
    (A/B = msg_w1 row blocks; the edge-feature rows multiply zeros).
  - relu(u+v') == max(u, -v') - (-v'), so with W := -v' the aggregated message
    S_i = sum_{j!=i} relu(u_i + v'_j)
        = sum_j max(u_i, W_j) - max(u_i, W_i) + W_i - sum_j W_j
    The max table is one DVE op per chunk (GPSIMD pre-expands u along j so both
    operands stream at the DVE 2x fp16 rate), reduced over j by a 3-level
    tensor_tensor adder tree; every linear term is folded into the
    PSUM-accumulated m_agg matmuls with +/-msg_w2.
  - sum_j W_j comes from the per-graph pooled h (already needed for the logits
    head): sum_j v'_j = B^T pooled_h + 8 b1.
  - Node MLP: psum = C2^T h + C3^T m_agg + I @ xc (xc = C1^T x + node_b1 is
    step-invariant), relu on ACT, then layer 2. Logits head on pooled h_new.
"""

import sys

if "/opt/trn_rl_repo" not in sys.path:
    sys.path.insert(0, "/opt/trn_rl_repo")

from contextlib import ExitStack

import numpy as np

import concourse.bass as bass  # noqa: F401
import concourse.mybir as mybir
import concourse.tile as tile
from concourse import bacc
from concourse.bass_utils import run_bass_kernel_spmd
from concourse.masks import make_identity

N_CORES = 8
BS = 2048
N_NODES = 8
N_STEPS = 8
G = BS // N_CORES          # graphs per core
NCOL = G * N_NODES         # node columns per core
CH = 4                     # chunks per step
CW = NCOL // CH            # node cols per chunk (512)
GW = G // CH               # graphs per chunk (64)
PW = GW * 64               # pair cols per chunk (4096)
F32 = mybir.dt.float32
F16 = mybir.dt.float16
AF = mybir.ActivationFunctionType
ALU = mybir.AluOpType

_W_NAMES = [
    ("pre_w1", [128, 128]),
    ("pre_w2", [128, 128]),
    ("msg_a", [128, 128]),
    ("msg_b", [128, 128]),
    ("msg_w2", [128, 128]),
    ("c1", [128, 128]),
    ("c2", [128, 128]),
    ("c3", [128, 128]),
    ("node_w2", [128, 128]),
    ("out_w1", [128, 128]),
    ("out_w2", [128, 100]),
]
_B_NAMES = [
    ("pre_b1", 128),
    ("pre_b2", 128),
    ("msg_b1", 128),
    ("msg_b2", 128),
    ("node_b1", 128),
    ("node_b2", 128),
    ("out_b1", 128),
    ("out_b2", 100),
]


def _kernel_body(tc, d):
    nc = tc.nc
    mm = nc.tensor.matmul
    with ExitStack() as ctx:
        wp = ctx.enter_context(tc.tile_pool(name="wp", bufs=1))
        sp = ctx.enter_context(tc.tile_pool(name="sp", bufs=1))
        rot = ctx.enter_context(tc.tile_pool(name="rot", bufs=2))
        mxp = ctx.enter_context(tc.tile_pool(name="mxp", bufs=2))
        ps = ctx.enter_context(tc.tile_pool(name="ps", bufs=1, space="PSUM"))

        # ---- resident weights (fp16) / biases (fp32) -----------------------
        w = {}
        for name, shape in _W_NAMES:
            w[name] = wp.tile(shape, F16, name=f"w_{name}", tag=f"w_{name}")
            nc.sync.dma_start(w[name][:], d[name].ap())
        b = {}
        for name, n in _B_NAMES:
            b[name] = wp.tile([128, 1], F32, name=f"b_{name}", tag=f"b_{name}")
            nc.sync.dma_start(b[name][:n, :], d[name].ap()[:, None])

        ident = wp.tile([128, 128], F16, name="ident", tag="ident")
        make_identity(nc, ident[:])

        # derived small constants
        w2n = wp.tile([128, 128], F16, name="w2n", tag="w2n")
        nc.scalar.mul(w2n[:], w["msg_w2"][:], -1.0)
        b2x7 = wp.tile([128, 1], F32, name="b2x7", tag="b2x7")
        nc.scalar.mul(b2x7[:], b["msg_b2"][:], 7.0)

        iota_i = wp.tile([128, 1], mybir.dt.int32, name="iota_i", tag="iota_i")
        nc.gpsimd.iota(iota_i[:], pattern=[[0, 1]], base=0, channel_multiplier=1)
        iota_f = wp.tile([128, 1], F32, name="iota_f", tag="iota_f")
        nc.vector.tensor_copy(iota_f[:], iota_i[:])

        # ---- pre phase: one-hot features + pre-MLP -------------------------
        idxb = sp.tile([128, NCOL], F32, name="idxb", tag="idxb")
        nc.sync.dma_start(idxb[:], d["idxb"].ap())
        moh = sp.tile([128, NCOL], F16, name="moh", tag="moh")
        l1 = sp.tile([128, NCOL], F16, name="l1", tag="l1")
        x = sp.tile([128, NCOL], F16, name="x", tag="x")
        xc = sp.tile([128, NCOL], F16, name="xc", tag="xc")
        for c in range(CH):
            cs = slice(c * CW, (c + 1) * CW)
            nc.vector.tensor_scalar(
                moh[:, cs], idxb[:, cs], iota_f[:], None, op0=ALU.is_equal
            )
            pp = ps.tile([128, CW], F32, name="ps", tag="ps", bufs=2)
            mm(out=pp[:], lhsT=w["pre_w1"][:], rhs=moh[:, cs])
            nc.scalar.activation(l1[:, cs], pp[:], AF.Relu, bias=b["pre_b1"][:])
            pp2 = ps.tile([128, CW], F32, name="ps", tag="ps", bufs=2)
            mm(out=pp2[:], lhsT=w["pre_w2"][:], rhs=l1[:, cs])
            nc.scalar.activation(x[:, cs], pp2[:], AF.Identity, bias=b["pre_b2"][:])
            pp3 = ps.tile([128, CW], F32, name="ps", tag="ps", bufs=2)
            mm(out=pp3[:], lhsT=w["c1"][:], rhs=x[:, cs])
            nc.scalar.activation(xc[:, cs], pp3[:], AF.Identity, bias=b["node_b1"][:])

        pooled = rot.tile([128, G], F16, name="pooled", tag="pooled")
        with nc.allow_low_precision("fp16 pooled sum of 8"):
            nc.vector.reduce_sum(
                pooled[:], x[:].rearrange("p (g i) -> p g i", i=N_NODES),
                axis=mybir.AxisListType.X,
            )

        hbufs = [sp.tile([128, NCOL], F16, name=f"h{k}", tag=f"h{k}") for k in range(2)]

        # ---- recurrent steps ----------------------------------------------
        # Pairwise pre-activations z_ij = u_i + v'_j are built by the PE
        # itself: two PSUM-accumulated matmuls whose rhs APs broadcast the
        # node columns across j (A side) / across i (B side). relu(+b1) then
        # lands the pair table in SBUF fp16 (split across ACT and DVE), and a
        # 3-op adder tree reduces over j.
        GZ = 16                      # graphs per z-psum tile (1024 pairs)
        NZT = G // GZ                # z tiles per step (16)
        for s in range(N_STEPS):
            h = x if s == 0 else hbufs[(s + 1) % 2]
            hn = hbufs[s % 2]
            r = sp.tile([128, NCOL], F16, name="r", tag="r")
            mdr = sp.tile([128, NCOL], F16, name="mdr", tag="mdr")
            magg = sp.tile([128, NCOL], F16, name="magg", tag="magg")
            nl1 = sp.tile([128, NCOL], F16, name="nl1", tag="nl1")
            mx = mxp.tile([128, NCOL * N_NODES], F16, name="mx", tag="mx")
            h3 = h[:].rearrange("p (g i) -> p g i", i=N_NODES)

            for t in range(NZT):
                zp = ps.tile([128, 2 * CW], F32, name="zps", tag="zps", bufs=3)
                for q in range(2):
                    g0 = t * GZ + q * 8
                    rhs_a = h3[:, g0:g0 + 8, :, None].to_broadcast(
                        [128, 8, N_NODES, N_NODES])
                    zsl = zp[:, q * CW:(q + 1) * CW]
                    mm(out=zsl, lhsT=w["msg_a"][:], rhs=rhs_a,
                       start=True, stop=False)
                for q in range(2):
                    g0 = t * GZ + q * 8
                    rhs_b = h3[:, g0:g0 + 8, None, :].to_broadcast(
                        [128, 8, N_NODES, N_NODES])
                    zsl = zp[:, q * CW:(q + 1) * CW]
                    mm(out=zsl, lhsT=w["msg_b"][:], rhs=rhs_b,
                       start=False, stop=True)
                msl = mx[:, t * 2 * CW:(t + 1) * 2 * CW]
                nc.scalar.activation(msl, zp[:], AF.Relu, bias=b["msg_b1"][:])

            # diagonal term relu(u_i + v'_i) and adder tree over j
            for c in range(CH):
                cs = slice(c * CW, (c + 1) * CW)
                zd = ps.tile([128, CW], F32, name="ps", tag="ps", bufs=2)
                mm(out=zd[:], lhsT=w["msg_a"][:], rhs=h[:, cs],
                   start=True, stop=False)
                mm(out=zd[:], lhsT=w["msg_b"][:], rhs=h[:, cs],
                   start=False, stop=True)
                nc.scalar.activation(mdr[:, cs], zd[:], AF.Relu, bias=b["msg_b1"][:])

            mx3 = mx[:].rearrange("p (n j) -> p n j", j=N_NODES)
            for c in range(CH):
                ns = slice(c * CW, (c + 1) * CW)
                t1 = mxp.tile([128, CW, 4], F16, name="t1", tag="t1")
                nc.vector.tensor_add(t1[:], mx3[:, ns, 0:4], mx3[:, ns, 4:8])
                t2 = mxp.tile([128, CW, 2], F16, name="t2", tag="t2")
                nc.vector.tensor_add(t2[:], t1[:, :, 0:2], t1[:, :, 2:4])
                nc.vector.tensor_add(
                    r[:, ns].rearrange("p (n one) -> p n one", one=1),
                    t2[:, :, 0:1], t2[:, :, 1:2],
                )

            # m_agg = W2^T (R - diag) + 7 b2
            for c in range(CH):
                cs = slice(c * CW, (c + 1) * CW)
                pm = ps.tile([128, CW], F32, name="ps", tag="ps", bufs=2)
                mm(out=pm[:], lhsT=w["msg_w2"][:], rhs=r[:, cs],
                   start=True, stop=False)
                mm(out=pm[:], lhsT=w2n[:], rhs=mdr[:, cs], start=False, stop=True)
                nc.scalar.activation(magg[:, cs], pm[:], AF.Identity, bias=b2x7[:])

                # node MLP layer 1: C2^T h + C3^T m_agg + xc, relu
                pn = ps.tile([128, CW], F32, name="ps", tag="ps", bufs=2)
                mm(out=pn[:], lhsT=w["c2"][:], rhs=h[:, cs], start=True, stop=False)
                mm(out=pn[:], lhsT=w["c3"][:], rhs=magg[:, cs],
                   start=False, stop=False)
                mm(out=pn[:], lhsT=ident[:], rhs=xc[:, cs], start=False, stop=True)
                nc.scalar.activation(nl1[:, cs], pn[:], AF.Relu)

                # node MLP layer 2
                ph = ps.tile([128, CW], F32, name="ps", tag="ps", bufs=2)
                mm(out=ph[:], lhsT=w["node_w2"][:], rhs=nl1[:, cs])
                nc.scalar.activation(
                    hn[:, cs], ph[:], AF.Identity, bias=b["node_b2"][:]
                )

            pooled = rot.tile([128, G], F16, name="pooled", tag="pooled")
            with nc.allow_low_precision("fp16 pooled sum of 8"):
                nc.vector.reduce_sum(
                    pooled[:], hn[:].rearrange("p (g i) -> p g i", i=N_NODES),
                    axis=mybir.AxisListType.X,
                )

            # logits head on pooled h_new
            po1 = ps.tile([128, G], F32, name="pst", tag="ps", bufs=2)
            mm(out=po1[:], lhsT=w["out_w1"][:], rhs=pooled[:])
            o1 = rot.tile([128, G], F16, name="o1", tag="o1")
            nc.scalar.activation(o1[:], po1[:], AF.Relu, bias=b["out_b1"][:])
            po2 = ps.tile([128, G], F32, name="pst", tag="ps", bufs=2)
            mm(out=po2[:100], lhsT=w["out_w2"][:], rhs=o1[:])
            olog = rot.tile([128, G], F32, name="olog", tag="olog")
            nc.scalar.activation(
                olog[:100, :], po2[:100], AF.Identity, bias=b["out_b2"][:100]
            )
            nc.sync.dma_start(d["out"].ap()[s], olog[:100, :])


def build_nc():
    nc = bacc.Bacc(
        "TRN2",
        target_bir_lowering=False,
        debug=False,
        enable_asserts=True,
        num_devices=N_CORES,
    )
    d = {}
    d["idxb"] = nc.dram_tensor("idxb", [128, NCOL], F32, kind="ExternalInput")
    for name, shape in _W_NAMES:
        d[name] = nc.dram_tensor(name, shape, F16, kind="ExternalInput")
    for name, n in _B_NAMES:
        d[name] = nc.dram_tensor(name, [n], F32, kind="ExternalInput")
    d["out"] = nc.dram_tensor(
        "out", [N_STEPS, 100, G], F32, kind="ExternalOutput"
    )
    with tile.TileContext(nc) as tc:
        _kernel_body(tc, d)
    nc.compile()
    return nc


def prep_inputs(**inputs):
    """Host-side layout prep: shard + transpose/slice/cast. Returns in_maps."""
    f32 = np.float32
    f16 = np.float16
    src = np.asarray(inputs["sources"]).astype(f32)        # (BS, 8)
    tgt = np.asarray(inputs["targets"]).astype(f32)
    typ = np.asarray(inputs["types"]).astype(f32)
    dif = np.asarray(inputs["diffs"]).astype(f32)
    q = np.asarray(inputs["question"]).astype(f32)         # (BS,)
    qrep = np.repeat(q[:, None], N_NODES, axis=1)

    shared = {}
    pre_w1 = np.asarray(inputs["pre_w1"], dtype=f32)
    pre_w1p = np.zeros((128, 128), f32)
    pre_w1p[:127] = pre_w1
    shared["pre_w1"] = pre_w1p
    shared["pre_w2"] = np.asarray(inputs["pre_w2"], dtype=f32)
    msg_w1 = np.asarray(inputs["msg_w1"], dtype=f32)
    shared["msg_a"] = msg_w1[0:128]
    shared["msg_b"] = msg_w1[128:256]
    shared["msg_w2"] = np.asarray(inputs["msg_w2"], dtype=f32)
    node_w1 = np.asarray(inputs["node_w1"], dtype=f32)
    shared["c1"] = node_w1[0:128]
    shared["c2"] = node_w1[128:256]
    shared["c3"] = node_w1[256:384]
    shared["node_w2"] = np.asarray(inputs["node_w2"], dtype=f32)
    shared["out_w1"] = np.asarray(inputs["out_w1"], dtype=f32)
    shared["out_w2"] = np.asarray(inputs["out_w2"], dtype=f32)
    shared = {k: np.ascontiguousarray(v, dtype=f16) for k, v in shared.items()}
    for name, _n in _B_NAMES:
        shared[name] = np.ascontiguousarray(inputs[name], dtype=f32)

    # per-partition-group index rows (offsets into the 127 one-hot rows)
    rows = [src, 8.0 + tgt, 16.0 + typ, 19.0 + dif, 119.0 + qrep]
    bounds = [(0, 8), (8, 16), (16, 19), (19, 119), (119, 127)]
    in_maps = []
    for c in range(N_CORES):
        gsl = slice(c * G, (c + 1) * G)
        idxb = np.full((128, NCOL), -1.0, f32)
        for (lo, hi), row in zip(bounds, rows):
            idxb[lo:hi, :] = row[gsl].reshape(-1)[None, :]
        m = dict(shared)
        m["idxb"] = idxb
        in_maps.append(m)
    return in_maps


_NC_CACHE = {}


def kernel(**inputs) -> np.ndarray:
    if "nc" not in _NC_CACHE:
        _NC_CACHE["nc"] = build_nc()
    nc = _NC_CACHE["nc"]
    in_maps = prep_inputs(**inputs)
    res = run_bass_kernel_spmd(nc, in_maps, list(range(N_CORES)))
    parts = [res.results[c]["out"].transpose(0, 2, 1) for c in range(N_CORES)]
    return np.ascontiguousarray(
        np.concatenate(parts, axis=1), dtype=np.float32
    )


# revision 12
# speedup vs baseline: 3.2836x; 1.1475x over previous
"""Trainium2 Bass kernel for nn_AgesRRN (batched 8-node GNN message passing).

Strategy (pure data parallel over 8 cores, 256 graphs each):
  - Activations live transposed in SBUF: [128 feature partitions x 2048 node cols],
    stored fp16 (values are O(1e-2); fp16 keeps ~5e-4 relative accuracy end to
    end while doubling DVE throughput and running the PE at 1 cycle/row).
  - Edges are ALL ordered pairs (i,j), i!=j inside each 8-node graph, so the
    edge-MLP layer 1 splits as u_i + v'_j with u = A^T h, v' = B^T h + b1
    (A/B = msg_w1 row blocks; the edge-feature rows multiply zeros).
  - relu(u+v') == max(u, -v') - (-v'), so with W := -v' the aggregated message
    S_i = sum_{j!=i} relu(u_i + v'_j)
        = sum_j max(u_i, W_j) - max(u_i, W_i) + W_i - sum_j W_j
    The max table is one DVE op per chunk (GPSIMD pre-expands u along j so both
    operands stream at the DVE 2x fp16 rate), reduced over j by a 3-level
    tensor_tensor adder tree; every linear term is folded into the
    PSUM-accumulated m_agg matmuls with +/-msg_w2.
  - sum_j W_j comes from the per-graph pooled h (already needed for the logits
    head): sum_j v'_j = B^T pooled_h + 8 b1.
  - Node MLP: psum = C2^T h + C3^T m_agg + I @ xc (xc = C1^T x + node_b1 is
    step-invariant), relu on ACT, then layer 2. Logits head on pooled h_new.
"""

import sys

if "/opt/trn_rl_repo" not in sys.path:
    sys.path.insert(0, "/opt/trn_rl_repo")

from contextlib import ExitStack

import numpy as np

import concourse.bass as bass  # noqa: F401
import concourse.mybir as mybir
import concourse.tile as tile
from concourse import bacc
from concourse.bass_utils import run_bass_kernel_spmd
from concourse.masks import make_identity

N_CORES = 8
BS = 2048
N_NODES = 8
N_STEPS = 8
G = BS // N_CORES          # graphs per core
NCOL = G * N_NODES         # node columns per core
CH = 4                     # chunks per step
CW = NCOL // CH            # node cols per chunk (512)
GW = G // CH               # graphs per chunk (64)
PW = GW * 64               # pair cols per chunk (4096)
F32 = mybir.dt.float32
F16 = mybir.dt.float16
AF = mybir.ActivationFunctionType
ALU = mybir.AluOpType

_W_NAMES = [
    ("pre_w1", [128, 128]),
    ("pre_w2", [128, 128]),
    ("msg_a", [128, 128]),
    ("msg_b", [128, 128]),
    ("msg_w2", [128, 128]),
    ("c1", [128, 128]),
    ("c2", [128, 128]),
    ("c3", [128, 128]),
    ("node_w2", [128, 128]),
    ("out_w1", [128, 128]),
    ("out_w2", [128, 100]),
]
_B_NAMES = [
    ("pre_b1", 128),
    ("pre_b2", 128),
    ("msg_b1", 128),
    ("msg_b2", 128),
    ("node_b1", 128),
    ("node_b2", 128),
    ("out_b1", 128),
    ("out_b2", 100),
]


def _kernel_body(tc, d):
    nc = tc.nc
    mm = nc.tensor.matmul
    with ExitStack() as ctx:
        wp = ctx.enter_context(tc.tile_pool(name="wp", bufs=1))
        sp = ctx.enter_context(tc.tile_pool(name="sp", bufs=1))
        rot = ctx.enter_context(tc.tile_pool(name="rot", bufs=2))
        mxp = ctx.enter_context(tc.tile_pool(name="mxp", bufs=2))
        ps = ctx.enter_context(tc.tile_pool(name="ps", bufs=1, space="PSUM"))

        # ---- resident weights (fp16) / biases (fp32) -----------------------
        w = {}
        for name, shape in _W_NAMES:
            w[name] = wp.tile(shape, F16, name=f"w_{name}", tag=f"w_{name}")
            nc.sync.dma_start(w[name][:], d[name].ap())
        b = {}
        for name, n in _B_NAMES:
            b[name] = wp.tile([128, 1], F32, name=f"b_{name}", tag=f"b_{name}")
            nc.sync.dma_start(b[name][:n, :], d[name].ap()[:, None])

        ident = wp.tile([128, 128], F16, name="ident", tag="ident")
        make_identity(nc, ident[:])

        # derived small constants
        w2n = wp.tile([128, 128], F16, name="w2n", tag="w2n")
        nc.scalar.mul(w2n[:], w["msg_w2"][:], -1.0)
        b2x7 = wp.tile([128, 1], F32, name="b2x7", tag="b2x7")
        nc.scalar.mul(b2x7[:], b["msg_b2"][:], 7.0)

        iota_i = wp.tile([128, 1], mybir.dt.int32, name="iota_i", tag="iota_i")
        nc.gpsimd.iota(iota_i[:], pattern=[[0, 1]], base=0, channel_multiplier=1)
        iota_f = wp.tile([128, 1], F32, name="iota_f", tag="iota_f")
        nc.vector.tensor_copy(iota_f[:], iota_i[:])

        # ---- pre phase: one-hot features + pre-MLP -------------------------
        idxb = sp.tile([128, NCOL], F32, name="idxb", tag="idxb")
        nc.sync.dma_start(idxb[:], d["idxb"].ap())
        moh = sp.tile([128, NCOL], F16, name="moh", tag="moh")
        l1 = sp.tile([128, NCOL], F16, name="l1", tag="l1")
        x = sp.tile([128, NCOL], F16, name="x", tag="x")
        xc = sp.tile([128, NCOL], F16, name="xc", tag="xc")
        for c in range(CH):
            cs = slice(c * CW, (c + 1) * CW)
            nc.vector.tensor_scalar(
                moh[:, cs], idxb[:, cs], iota_f[:], None, op0=ALU.is_equal
            )
            pp = ps.tile([128, CW], F32, name="ps", tag="ps", bufs=2)
            mm(out=pp[:], lhsT=w["pre_w1"][:], rhs=moh[:, cs])
            nc.scalar.activation(l1[:, cs], pp[:], AF.Relu, bias=b["pre_b1"][:])
            pp2 = ps.tile([128, CW], F32, name="ps", tag="ps", bufs=2)
            mm(out=pp2[:], lhsT=w["pre_w2"][:], rhs=l1[:, cs])
            nc.scalar.activation(x[:, cs], pp2[:], AF.Identity, bias=b["pre_b2"][:])
            pp3 = ps.tile([128, CW], F32, name="ps", tag="ps", bufs=2)
            mm(out=pp3[:], lhsT=w["c1"][:], rhs=x[:, cs])
            nc.scalar.activation(xc[:, cs], pp3[:], AF.Identity, bias=b["node_b1"][:])

        pooled = rot.tile([128, G], F16, name="pooled", tag="pooled")
        with nc.allow_low_precision("fp16 pooled sum of 8"):
            nc.vector.reduce_sum(
                pooled[:], x[:].rearrange("p (g i) -> p g i", i=N_NODES),
                axis=mybir.AxisListType.X,
            )

        hbufs = [sp.tile([128, NCOL], F16, name=f"h{k}", tag=f"h{k}") for k in range(2)]

        # ---- recurrent steps ----------------------------------------------
        # Pairwise pre-activations z_ij = u_i + v'_j are built by the PE
        # itself: two PSUM-accumulated matmuls whose rhs APs broadcast the
        # node columns across j (A side) / across i (B side). relu(+b1) then
        # lands the pair table in SBUF fp16 (split across ACT and DVE), and a
        # 3-op adder tree reduces over j.
        GZ = 16                      # graphs per z-psum tile (1024 pairs)
        NZT = G // GZ                # z tiles per step (16)
        for s in range(N_STEPS):
            h = x if s == 0 else hbufs[(s + 1) % 2]
            hn = hbufs[s % 2]
            r = sp.tile([128, NCOL], F16, name="r", tag="r")
            magg = sp.tile([128, NCOL], F16, name="magg", tag="magg")
            nl1 = sp.tile([128, NCOL], F16, name="nl1", tag="nl1")
            mx = mxp.tile([128, NCOL * N_NODES], F16, name="mx", tag="mx")
            h3 = h[:].rearrange("p (g i) -> p g i", i=N_NODES)

            for t in range(NZT):
                zp = ps.tile([128, 2 * CW], F32, name="zps", tag="zps", bufs=3)
                for q in range(2):
                    g0 = t * GZ + q * 8
                    rhs_a = h3[:, g0:g0 + 8, :, None].to_broadcast(
                        [128, 8, N_NODES, N_NODES])
                    zsl = zp[:, q * CW:(q + 1) * CW]
                    mm(out=zsl, lhsT=w["msg_a"][:], rhs=rhs_a,
                       start=True, stop=False)
                for q in range(2):
                    g0 = t * GZ + q * 8
                    rhs_b = h3[:, g0:g0 + 8, None, :].to_broadcast(
                        [128, 8, N_NODES, N_NODES])
                    zsl = zp[:, q * CW:(q + 1) * CW]
                    mm(out=zsl, lhsT=w["msg_b"][:], rhs=rhs_b,
                       start=False, stop=True)
                msl = mx[:, t * 2 * CW:(t + 1) * 2 * CW]
                if t % 8 == 3:
                    nc.vector.tensor_scalar(
                        msl, zp[:], b["msg_b1"][:], 0.0, op0=ALU.add, op1=ALU.max
                    )
                else:
                    nc.scalar.activation(msl, zp[:], AF.Relu, bias=b["msg_b1"][:])

            mx3 = mx[:].rearrange("p (n j) -> p n j", j=N_NODES)
            mx4 = mx[:].rearrange("p (g y) -> p g y", y=64)
            for c in range(CH):
                ns = slice(c * CW, (c + 1) * CW)
                gs = slice(c * GW, (c + 1) * GW)
                t1 = mxp.tile([128, CW, 4], F16, name="t1", tag="t1")
                nc.vector.tensor_add(t1[:], mx3[:, ns, 0:4], mx3[:, ns, 4:8])
                t2 = mxp.tile([128, CW, 2], F16, name="t2", tag="t2")
                nc.vector.tensor_add(t2[:], t1[:, :, 0:2], t1[:, :, 2:4])
                rs = mxp.tile([128, CW], F16, name="rs", tag="rs")
                nc.vector.tensor_add(
                    rs[:].rearrange("p (n one) -> p n one", one=1),
                    t2[:, :, 0:1], t2[:, :, 1:2],
                )
                # subtract the j==i diagonal (stride-9 within each graph block)
                nc.vector.tensor_sub(
                    r[:, ns].rearrange("p (g i) -> p g i", i=N_NODES),
                    rs[:].rearrange("p (g i) -> p g i", i=N_NODES),
                    mx4[:, gs, 0:64:9],
                )

            # m_agg = W2^T r + 7 b2
            for c in range(CH):
                cs = slice(c * CW, (c + 1) * CW)
                pm = ps.tile([128, CW], F32, name="ps", tag="ps", bufs=2)
                mm(out=pm[:], lhsT=w["msg_w2"][:], rhs=r[:, cs])
                nc.scalar.activation(magg[:, cs], pm[:], AF.Identity, bias=b2x7[:])

                # node MLP layer 1: C2^T h + C3^T m_agg + xc, relu
                pn = ps.tile([128, CW], F32, name="ps", tag="ps", bufs=2)
                mm(out=pn[:], lhsT=w["c2"][:], rhs=h[:, cs], start=True, stop=False)
                mm(out=pn[:], lhsT=w["c3"][:], rhs=magg[:, cs],
                   start=False, stop=False)
                mm(out=pn[:], lhsT=ident[:], rhs=xc[:, cs], start=False, stop=True)
                nc.vector.tensor_scalar(
                    nl1[:, cs], pn[:], 0.0, None, op0=ALU.max
                )

                # node MLP layer 2
                ph = ps.tile([128, CW], F32, name="ps", tag="ps", bufs=2)
                mm(out=ph[:], lhsT=w["node_w2"][:], rhs=nl1[:, cs])
                nc.scalar.activation(
                    hn[:, cs], ph[:], AF.Identity, bias=b["node_b2"][:]
                )

            pooled = rot.tile([128, G], F16, name="pooled", tag="pooled")
            with nc.allow_low_precision("fp16 pooled sum of 8"):
                nc.vector.reduce_sum(
                    pooled[:], hn[:].rearrange("p (g i) -> p g i", i=N_NODES),
                    axis=mybir.AxisListType.X,
                )

            # logits head on pooled h_new
            po1 = ps.tile([128, G], F32, name="pst", tag="ps", bufs=2)
            mm(out=po1[:], lhsT=w["out_w1"][:], rhs=pooled[:])
            o1 = rot.tile([128, G], F16, name="o1", tag="o1")
            nc.scalar.activation(o1[:], po1[:], AF.Relu, bias=b["out_b1"][:])
            po2 = ps.tile([128, G], F32, name="pst", tag="ps", bufs=2)
            mm(out=po2[:100], lhsT=w["out_w2"][:], rhs=o1[:])
            olog = rot.tile([128, G], F32, name="olog", tag="olog")
            nc.scalar.activation(
                olog[:100, :], po2[:100], AF.Identity, bias=b["out_b2"][:100]
            )
            nc.sync.dma_start(d["out"].ap()[s], olog[:100, :])


def build_nc():
    nc = bacc.Bacc(
        "TRN2",
        target_bir_lowering=False,
        debug=False,
        enable_asserts=True,
        num_devices=N_CORES,
    )
    d = {}
    d["idxb"] = nc.dram_tensor("idxb", [128, NCOL], F32, kind="ExternalInput")
    for name, shape in _W_NAMES:
        d[name] = nc.dram_tensor(name, shape, F16, kind="ExternalInput")
    for name, n in _B_NAMES:
        d[name] = nc.dram_tensor(name, [n], F32, kind="ExternalInput")
    d["out"] = nc.dram_tensor(
        "out", [N_STEPS, 100, G], F32, kind="ExternalOutput"
    )
    with tile.TileContext(nc) as tc:
        _kernel_body(tc, d)
    nc.compile()
    return nc


def prep_inputs(**inputs):
    """Host-side layout prep: shard + transpose/slice/cast. Returns in_maps."""
    f32 = np.float32
    f16 = np.float16
    src = np.asarray(inputs["sources"]).astype(f32)        # (BS, 8)
    tgt = np.asarray(inputs["targets"]).astype(f32)
    typ = np.asarray(inputs["types"]).astype(f32)
    dif = np.asarray(inputs["diffs"]).astype(f32)
    q = np.asarray(inputs["question"]).astype(f32)         # (BS,)
    qrep = np.repeat(q[:, None], N_NODES, axis=1)

    shared = {}
    pre_w1 = np.asarray(inputs["pre_w1"], dtype=f32)
    pre_w1p = np.zeros((128, 128), f32)
    pre_w1p[:127] = pre_w1
    shared["pre_w1"] = pre_w1p
    shared["pre_w2"] = np.asarray(inputs["pre_w2"], dtype=f32)
    msg_w1 = np.asarray(inputs["msg_w1"], dtype=f32)
    shared["msg_a"] = msg_w1[0:128]
    shared["msg_b"] = msg_w1[128:256]
    shared["msg_w2"] = np.asarray(inputs["msg_w2"], dtype=f32)
    node_w1 = np.asarray(inputs["node_w1"], dtype=f32)
    shared["c1"] = node_w1[0:128]
    shared["c2"] = node_w1[128:256]
    shared["c3"] = node_w1[256:384]
    shared["node_w2"] = np.asarray(inputs["node_w2"], dtype=f32)
    shared["out_w1"] = np.asarray(inputs["out_w1"], dtype=f32)
    shared["out_w2"] = np.asarray(inputs["out_w2"], dtype=f32)
    shared = {k: np.ascontiguousarray(v, dtype=f16) for k, v in shared.items()}
    for name, _n in _B_NAMES:
        shared[name] = np.ascontiguousarray(inputs[name], dtype=f32)

    # per-partition-group index rows (offsets into the 127 one-hot rows)
    rows = [src, 8.0 + tgt, 16.0 + typ, 19.0 + dif, 119.0 + qrep]
    bounds = [(0, 8), (8, 16), (16, 19), (19, 119), (119, 127)]
    in_maps = []
    for c in range(N_CORES):
        gsl = slice(c * G, (c + 1) * G)
        idxb = np.full((128, NCOL), -1.0, f32)
        for (lo, hi), row in zip(bounds, rows):
            idxb[lo:hi, :] = row[gsl].reshape(-1)[None, :]
        m = dict(shared)
        m["idxb"] = idxb
        in_maps.append(m)
    return in_maps


_NC_CACHE = {}


def kernel(**inputs) -> np.ndarray:
    if "nc" not in _NC_CACHE:
        _NC_CACHE["nc"] = build_nc()
    nc = _NC_CACHE["nc"]
    in_maps = prep_inputs(**inputs)
    res = run_bass_kernel_spmd(nc, in_maps, list(range(N_CORES)))
    parts = [res.results[c]["out"].transpose(0, 2, 1) for c in range(N_CORES)]
    return np.ascontiguousarray(
        np.concatenate(parts, axis=1), dtype=np.float32
    )
